# revision 2
# baseline (speedup 1.0000x reference)
"""Multi-LOD dense-grid trilinear interpolation on 8 trn2 cores.

Strategy (data-parallel over points, grids replicated):
  - Host prepacks each LOD grid into a "G4" layout: row (x,y,z) holds the
    4 (x,y)-corner features [G[x,y,z], G[x+1,y,z], G[x,y+1,z], G[x+1,y+1,z]]
    (clamped at the upper faces) as 16 bf16 = 32B. One 64B indirect-DMA
    descriptor per (point, LOD) then fetches rows z0,z0+1 = all 8 corners.
  - One multi-offset indirect DMA per (tile, LOD) carries 128*E descriptors,
    amortizing the ~1us SWDGE fixed cost.
  - Points and outputs travel in the on-chip layout (host pre/post permute).
  - Weights/reduction on DVE in bf16; coords in f32 on DVE+Act.

kernel(**inputs) takes FULL inputs, returns the FULL [N, 20] output.
"""

import math
import numpy as np

import concourse.bass as bass
import concourse.bacc as bacc
import concourse.mybir as mybir
import concourse.tile as tile
from concourse import bass_utils

P = 128
NUM_LODS = 5
FEAT = 4
LODS = [16, 32, 64, 128, 256]
N_PTS = 1_000_000
N_CORES = 8
OUT_D = NUM_LODS * FEAT  # 20

f32 = mybir.dt.float32
i32 = mybir.dt.int32
bf16 = mybir.dt.bfloat16
BF16_NP = mybir.dt.np(bf16)

E = 128  # point-columns per tile (tile = P*E = 16384 points)
DMA_SCRATCH = 65536  # SWDGE descriptor-ring carveout (bytes)

ROBUST_FLOOR = True  # i32 conversion rounds -> need is_gt correction


def _v(t_ap: bass.AP, off_elems: int, dims) -> bass.AP:
    part = [list(t_ap.ap[0])[0], list(t_ap.ap[0])[1]]
    return bass.AP(
        t_ap.tensor,
        t_ap.offset + off_elems,
        [part] + [[int(s), int(c)] for s, c in dims],
    )


def n_tiles_per_core(n: int = N_PTS) -> int:
    n_per_core = math.ceil(n / N_CORES)
    return math.ceil(n_per_core / (P * E))


def build_kernel(tc, out_ap, pts_ap, grid_aps, n_tiles, repeats=1):
    nc = tc.nc
    with (
        tc.tile_pool(name="io", bufs=2) as io_pool,
        tc.tile_pool(name="sm", bufs=2) as sm_pool,
        tc.tile_pool(name="gd", bufs=7) as gd_pool,
    ):
        for rep in range(repeats):
            for ti in range(n_tiles):
                sfx = f"_{rep}_{ti}"
                # pts tile: [p, e*3+c] (contig slice of the prepacked layout)
                pts_t = io_pool.tile([P, 3 * E], f32, tag="pts", name="pts" + sfx)
                nc.sync.dma_start(
                    pts_t, _v(pts_ap, ti * 3 * E, [[1, 3 * E]])
                )
                out_t = io_pool.tile([P, OUT_D * E], f32, tag="out", name="out" + sfx)

                gd_ts = []
                w8_ts = []
                # ---- phase A: coords, weights, gather launch (all LODs) ----
                for l, r in enumerate(LODS):
                    lsfx = f"{sfx}_{l}"
                    c_t = sm_pool.tile([P, 3 * E], f32, tag="c", name="c" + lsfx)
                    nc.scalar.mul(c_t, pts_t, float(r - 1))
                    ii_t = sm_pool.tile([P, 3 * E], i32, tag="ii", name="ii" + lsfx)
                    nc.vector.tensor_copy(ii_t, c_t)
                    i0_t = sm_pool.tile([P, 3 * E], f32, tag="i0", name="i0" + lsfx)
                    nc.vector.tensor_copy(i0_t, ii_t)
                    if ROBUST_FLOOR:
                        cmp_t = sm_pool.tile(
                            [P, 3 * E], f32, tag="cmp", name="cmp" + lsfx
                        )
                        nc.vector.tensor_tensor(
                            cmp_t, i0_t, c_t, mybir.AluOpType.is_gt
                        )
                        nc.vector.tensor_tensor(
                            i0_t, i0_t, cmp_t, mybir.AluOpType.subtract
                        )
                    f_t = sm_pool.tile([P, 3 * E], f32, tag="f", name="f" + lsfx)
                    nc.vector.tensor_tensor(f_t, c_t, i0_t, mybir.AluOpType.subtract)

                    # ---- index: ((x0*r + y0)*r + z0) ----
                    t1_t = sm_pool.tile([P, E], f32, tag="t1", name="t1" + lsfx)
                    nc.vector.scalar_tensor_tensor(
                        t1_t, _v(i0_t, 0, [[3, E]]), float(r), _v(i0_t, 1, [[3, E]]),
                        mybir.AluOpType.mult, mybir.AluOpType.add,
                    )
                    idxf_t = sm_pool.tile([P, E], f32, tag="idxf", name="idxf" + lsfx)
                    nc.vector.scalar_tensor_tensor(
                        idxf_t, t1_t, float(r), _v(i0_t, 2, [[3, E]]),
                        mybir.AluOpType.mult, mybir.AluOpType.add,
                    )
                    idxi_t = sm_pool.tile([P, E], i32, tag="idxi", name="idxi" + lsfx)
                    nc.vector.tensor_copy(idxi_t, idxf_t)

                    # ---- weights: fg[e, axis*2 + sel]  (sel 0:1-f, 1:f) ----
                    fg_t = sm_pool.tile([P, 6 * E], f32, tag="fg", name="fg" + lsfx)
                    nc.vector.tensor_copy(
                        _v(fg_t, 1, [[6, E], [2, 3]]), _v(f_t, 0, [[3, E], [1, 3]])
                    )
                    nc.scalar.activation(
                        _v(fg_t, 0, [[6, E], [2, 3]]),
                        _v(f_t, 0, [[3, E], [1, 3]]),
                        mybir.ActivationFunctionType.Copy, bias=1.0, scale=-1.0,
                    )
                    # w4[e, y*2+x] = fg[e,1,y] * fg[e,0,x]
                    w4_t = sm_pool.tile([P, 4 * E], f32, tag="w4", name="w4" + lsfx)
                    nc.vector.tensor_tensor(
                        _v(w4_t, 0, [[4, E], [2, 2], [1, 2]]),
                        _v(fg_t, 0, [[6, E], [0, 2], [1, 2]]),
                        _v(fg_t, 2, [[6, E], [1, 2], [0, 2]]),
                        mybir.AluOpType.mult,
                    )
                    # w8[e, z*4+k] = w4[e,k] * fg[e,2,z]
                    w8_t = sm_pool.tile([P, 8 * E], bf16, tag="w8", name="w8" + lsfx)
                    nc.vector.tensor_tensor(
                        _v(w8_t, 0, [[8, E], [4, 2], [1, 4]]),
                        _v(w4_t, 0, [[4, E], [0, 2], [1, 4]]),
                        _v(fg_t, 4, [[6, E], [1, 2], [0, 4]]),
                        mybir.AluOpType.mult,
                    )
                    w8_ts.append(w8_t)

                    # ---- gather: per column, 128 descs of 2 G4 rows = 64B.
                    # (offset APs wider than [128,1] are mis-executed by the
                    # SWDGE ucode: only offset 0 is used, rest streams
                    # contiguously -- verified on HW. So one instr per column.)
                    gd_t = gd_pool.tile([P, 32 * E], bf16, tag="gd", name="gd" + lsfx)
                    for e in range(E):
                        nc.gpsimd.indirect_dma_start(
                            out=gd_t[:, e * 32: (e + 1) * 32],
                            out_offset=None,
                            in_=grid_aps[l],
                            in_offset=bass.IndirectOffsetOnAxis(
                                ap=idxi_t[:, e: e + 1], axis=0
                            ),
                            oob_is_err=False,
                        )
                    gd_ts.append(gd_t)

                # ---- phase B: weighted reduce (all LODs) ----
                for l, r in enumerate(LODS):
                    lsfx = f"{sfx}_{l}"
                    gd_t = gd_ts[l]
                    w8_t = w8_ts[l]
                    gw_t = sm_pool.tile([P, 32 * E], bf16, tag="gw", name="gw" + lsfx)
                    for z in range(2):
                        nc.vector.tensor_tensor(
                            _v(gw_t, z * 16, [[32, E], [4, 4], [1, 4]]),
                            _v(gd_t, z * 16, [[32, E], [4, 4], [1, 4]]),
                            _v(w8_t, z * 4, [[8, E], [1, 4], [0, 4]]),
                            mybir.AluOpType.mult,
                        )
                    # out[e, l*4+f] = sum_{zk} gw[e, zk*4+f]
                    nc.vector.tensor_reduce(
                        _v(out_t, l * 4, [[OUT_D, E], [1, 4]]),
                        _v(gw_t, 0, [[32, E], [1, 4], [4, 8]]),
                        mybir.AxisListType.X,
                        mybir.AluOpType.add,
                    )

                nc.sync.dma_start(
                    _v(out_ap, ti * OUT_D * E, [[1, OUT_D * E]]), out_t
                )


def _build_g4(grid: np.ndarray, r: int) -> np.ndarray:
    """Pack grid [r^3, F] f32 into G4 [(r^3+1), 16] bf16 rows."""
    G = np.asarray(grid, np.float32).reshape(r, r, r, FEAT)
    xp = np.minimum(np.arange(r) + 1, r - 1)
    out = np.empty((r * r * r + 1, 4 * FEAT), dtype=BF16_NP)
    v = out[: r * r * r].reshape(r, r, r, 4, FEAT)
    v[..., 0, :] = G.astype(BF16_NP)
    Gx = G[xp]
    v[..., 1, :] = Gx.astype(BF16_NP)
    v[..., 2, :] = G[:, xp].astype(BF16_NP)
    v[..., 3, :] = Gx[:, xp].astype(BF16_NP)
    out[r * r * r] = 0
    return out


def _pack_pts(pts: np.ndarray, lo: int, hi: int, n_tiles: int) -> np.ndarray:
    n_padded = n_tiles * P * E
    chunk = np.zeros((n_padded, 3), dtype=np.float32)
    chunk[: hi - lo] = pts[lo:hi]
    # row t*P*E + e*P + p  ->  ptsl[p, ((t*E)+e)*3+c]
    return np.ascontiguousarray(
        chunk.reshape(n_tiles, E, P, 3).transpose(2, 0, 1, 3).reshape(P, -1)
    )


def _unpack_out(core_out: np.ndarray, n_tiles: int) -> np.ndarray:
    # out_dev [P, T*E*20] -> [n_padded, 20]
    return (
        core_out.reshape(P, n_tiles, E, OUT_D)
        .transpose(1, 2, 0, 3)
        .reshape(n_tiles * P * E, OUT_D)
    )


_COMPILED = {}


def _get_compiled(n_tiles: int = None, repeats: int = 1):
    if n_tiles is None:
        n_tiles = n_tiles_per_core()
    key = (n_tiles, repeats)
    if key in _COMPILED:
        return _COMPILED[key]
    nc = bacc.Bacc(
        "TRN2", debug=False, enable_asserts=False,
        dynamic_dma_scratch_size=DMA_SCRATCH,
    )
    pts_ap = nc.dram_tensor(
        "pts", [P, n_tiles * 3 * E], f32, kind="ExternalInput"
    ).ap()
    grid_aps = [
        nc.dram_tensor(
            f"grid{l}", [LODS[l] ** 3 + 1, 4 * FEAT], bf16, kind="ExternalInput"
        ).ap()
        for l in range(NUM_LODS)
    ]
    out_ap = nc.dram_tensor(
        "out", [P, n_tiles * OUT_D * E], f32, kind="ExternalOutput"
    ).ap()
    with tile.TileContext(nc) as tc:
        build_kernel(tc, out_ap, pts_ap, grid_aps, n_tiles, repeats=repeats)
    nc.compile()
    _COMPILED[key] = nc
    return nc


def make_in_maps(pts, grids_np):
    """Host-side shard + prepack: per-core input dict list."""
    pts = np.ascontiguousarray(np.asarray(pts, dtype=np.float32))
    n = pts.shape[0]
    n_per_core = math.ceil(n / N_CORES)
    n_tiles = n_tiles_per_core(n)
    g4s = [_build_g4(grids_np[l], LODS[l]) for l in range(NUM_LODS)]
    in_maps = []
    for c in range(N_CORES):
        lo = c * n_per_core
        hi = min(n, (c + 1) * n_per_core)
        m = {"pts": _pack_pts(pts, lo, hi, n_tiles)}
        for l in range(NUM_LODS):
            m[f"grid{l}"] = g4s[l]
        in_maps.append(m)
    return in_maps, n_per_core, n_tiles


def kernel(pts, grid0, grid1, grid2, grid3, grid4, _trace=False, _tmpdir=None):
    grids = [grid0, grid1, grid2, grid3, grid4]
    in_maps, n_per_core, n_tiles = make_in_maps(pts, grids)
    n = np.asarray(pts).shape[0]

    nc = _get_compiled(n_tiles)
    res = bass_utils.run_bass_kernel_spmd(
        nc, in_maps, core_ids=list(range(N_CORES)), trace=_trace, tmpdir=_tmpdir
    )
    out = np.empty((n, OUT_D), dtype=np.float32)
    for c in range(N_CORES):
        lo = c * n_per_core
        hi = min(n, (c + 1) * n_per_core)
        out[lo:hi] = _unpack_out(res.results[c]["out"], n_tiles)[: hi - lo]
    kernel.last_results = res
    return out
